# revision 20
# baseline (speedup 1.0000x reference)
"""FM synth (2-op feedback FM) Trainium2 kernel.

Contract: kernel(**inputs) takes FULL unsharded inputs
  fm_params [524288, 1, 6] f32, f0_hz [524288, 1] f32, phase_state [524288, 4] f32
returns (audio [524288, 64] f32, phase_end [524288, 4] f32), matching reference.

Data-parallel over 8 NeuronCores (batch sharding). Per core: 65536 voices.

Math (per voice, 64 steps, all phases tracked in TURNS = radians/2pi):
  inc1 = f0c*r1/16000, inc2 = f0c*r2/16000   (turns/step)
  v1_t = start1' + (t+1)*inc1 ; v2 lags one step behind (skewed fusion)
  osc1 step t:   u1 = v1_t + m1;  o1 = sin(2*pi*wrap(u1));  m1' = fb1'*o1
  osc2 step t-1: u2 = v2_t + m2;  o2 = sin(2*pi*wrap(u2));
                 m2' = fb2'*o2 + d2'*o1_t
  wrap(u) = u - round_to_nearest_int(u)  (int32 cast is RN on DVE)
  audio[v, t-1] = o2 ; then RMS-normalize over the 64 samples.

Layout: interleaved oscillator pairs [128 partitions, F voices, 2] with
c=0 -> osc2, c=1 -> osc1, so one [128, 2F] instruction covers both oscs and
the ACT sin writes osc2 into its audio column and osc1 into a scratch
column of the same tile via one affine access pattern
(audio tile is [128, F, 65]: columns 0..63 audio, column 64 = o1 scratch).
"""
import sys, math
from contextlib import ExitStack

import numpy as np

sys.path.insert(0, "/opt/trn_rl_repo")

import concourse.bass as bass
import concourse.tile as tile
from concourse import mybir
from concourse.bass_utils import run_bass_kernel_spmd

FP = mybir.dt.float32
I32 = mybir.dt.int32
AF = mybir.ActivationFunctionType
OP = mybir.AluOpType

B = 524288
NCORES = 8
VC = B // NCORES          # 65536 voices per core
S = 64                    # samples per block
P = 128                   # partitions
F = 256                   # voices per partition per chunk
CHUNK = P * F             # 32768 voices per chunk
NCHUNK = VC // CHUNK      # 2 chunks per core
TWO_PI = 2.0 * math.pi
INV_SR = 1.0 / 16000.0


def split_multi_waits(nc, max_waits: int = 1):
    """This walrus build rejects >1 sync-wait on most instruction forms.
    Split extras into standalone InstEventSemaphore waits inserted before."""
    k = 0
    for f in nc.m.functions:
        for blk in f.blocks:
            out = []
            for inst in blk.instructions:
                si = inst.sync_info
                if si is not None and len(si.on_wait) > max_waits:
                    waits = list(si.on_wait)
                    for w in waits[:-max_waits]:
                        out.append(
                            mybir.InstEventSemaphore(
                                name=f"splitw-{k}",
                                engine=inst.engine,
                                ins=[],
                                outs=[],
                                sync_info=mybir.SyncInfo(on_wait=[w], on_update=[]),
                            )
                        )
                        k += 1
                    inst.sync_info = mybir.SyncInfo(
                        on_wait=waits[-max_waits:], on_update=list(si.on_update)
                    )
                out.append(inst)
            blk.instructions = out
    return k


F16 = mybir.dt.float16
SBK = 16                   # sample-block size for RMS squaring
NBLK = S // SBK


def _build():
    nc = bass.Bass()
    fm_in = nc.declare_dram_parameter("fm", [VC, 6], FP, isOutput=False)
    f0_in = nc.declare_dram_parameter("f0", [VC, 1], FP, isOutput=False)
    ps_in = nc.declare_dram_parameter("ps", [VC, 4], FP, isOutput=False)
    audio_out = nc.declare_dram_parameter("audio", [VC, S], FP, isOutput=True)
    pe_out = nc.declare_dram_parameter("pe", [VC, 4], FP, isOutput=True)

    with ExitStack() as ctx:
        tc = ctx.enter_context(tile.TileContext(nc))
        io_pool = ctx.enter_context(tc.tile_pool(name="io", bufs=2))
        par_pool = ctx.enter_context(tc.tile_pool(name="par", bufs=1))
        one_pool = ctx.enter_context(tc.tile_pool(name="one", bufs=1))
        big_pool = ctx.enter_context(tc.tile_pool(name="big", bufs=1))
        scr_pool = ctx.enter_context(tc.tile_pool(name="scr", bufs=3))

        VINC = par_pool.tile([P, F, 8], FP, tag="VINC")

        def setup(c):
            lo = c * CHUNK
            hi = lo + CHUNK
            st = {"lo": lo, "hi": hi, "c": c}
            # VINC shared across chunks so the per-step V += INC fuses into
            # one wide op; FB/M per chunk (pairs: 0=osc2, 1=osc1)
            INC = VINC[:, :, 2 * c : 2 * c + 2]
            V = VINC[:, :, 4 + 2 * c : 6 + 2 * c]
            # cols: fb2 0 | fb1 1 | d2 2 | M2 3 | M1 4 | T2 5  (so that
            # [fb1,d2] and [M2,M1] and [M1,T2] are contiguous pair-slices)
            PARAMS = par_pool.tile([P, F, 8], FP, tag=f"PARAMS_{c}")
            FB = PARAMS[:, :, 0:2]          # [fb2, fb1]
            M = PARAMS[:, :, 3:5]           # [M2, M1] (0=osc2, 1=osc1)
            PARC = par_pool.tile([P, F, 4], FP, tag=f"PARC_{c}")
            ss, rr = PARC[:, :, 1], PARC[:, :, 2]
            D2 = PARAMS[:, :, 2]
            ps4 = par_pool.tile([P, F, 4], FP, tag=f"ps4_{c}")
            pe4 = par_pool.tile([P, F, 4], FP, tag=f"pe4_{c}")
            ONE = one_pool.tile([P, F, 12], FP, tag="ONE")
            f0t, f0s, r1, r2 = ONE[:, :, 0], ONE[:, :, 1], ONE[:, :, 2], ONE[:, :, 3]
            s1t, s2t, t0 = ONE[:, :, 4], ONE[:, :, 5], ONE[:, :, 6]
            wtmp, st_t = ONE[:, :, 7], ONE[:, :, 8]
            ktmp = ONE[:, :, 9:10].bitcast(I32)[:, :, 0]
            fm6 = io_pool.tile([P, F, 6], FP, tag="fm6")
            f0ld = io_pool.tile([P, F], FP, tag="f0ld")

            nc.gpsimd.dma_start(
                out=fm6[:], in_=fm_in[lo:hi, :].rearrange("(p f) c -> p f c", p=P)
            )
            nc.gpsimd.dma_start(
                out=ps4[:], in_=ps_in[lo:hi, :].rearrange("(p f) c -> p f c", p=P)
            )
            nc.gpsimd.dma_start(
                out=f0ld[:], in_=f0_in[lo:hi, 0].rearrange("(p f) -> p f", p=P)
            )
            nc.vector.tensor_copy(f0t, f0ld[:])

            nc.vector.tensor_scalar(out=f0s, in0=f0t, scalar1=1.0,
                                    scalar2=INV_SR, op0=OP.max, op1=OP.mult)
            nc.vector.tensor_scalar(out=r1, in0=fm6[:, :, 1], scalar1=15.75,
                                    scalar2=0.25, op0=OP.mult, op1=OP.add)
            nc.vector.tensor_scalar(out=r2, in0=fm6[:, :, 4], scalar1=15.75,
                                    scalar2=0.25, op0=OP.mult, op1=OP.add)

            nc.vector.tensor_mul(INC[:, :, 1], f0s, r1)
            nc.vector.tensor_mul(INC[:, :, 0], f0s, r2)
            nc.vector.tensor_scalar(out=FB[:, :, 1], in0=fm6[:, :, 2],
                                    scalar1=0.95 / TWO_PI, scalar2=None, op0=OP.mult)
            nc.vector.tensor_scalar(out=FB[:, :, 0], in0=fm6[:, :, 5],
                                    scalar1=0.95 / TWO_PI, scalar2=None, op0=OP.mult)
            nc.vector.tensor_scalar(out=D2, in0=fm6[:, :, 3],
                                    scalar1=10.0 / TWO_PI, scalar2=None, op0=OP.mult)

            # V init; loop does V += INC first, so pre-subtract
            nc.vector.tensor_scalar(out=s1t, in0=ps4[:, :, 0],
                                    scalar1=1.0 / TWO_PI, scalar2=None, op0=OP.mult)
            nc.vector.scalar_tensor_tensor(out=V[:, :, 1], in0=INC[:, :, 1],
                                           scalar=-1.0, in1=s1t,
                                           op0=OP.mult, op1=OP.add)
            nc.vector.tensor_scalar(out=s2t, in0=ps4[:, :, 1],
                                    scalar1=1.0 / TWO_PI, scalar2=None, op0=OP.mult)
            nc.vector.scalar_tensor_tensor(out=V[:, :, 0], in0=INC[:, :, 0],
                                           scalar=-2.0, in1=s2t,
                                           op0=OP.mult, op1=OP.add)

            nc.vector.tensor_mul(M[:, :, 1], FB[:, :, 1], ps4[:, :, 2])
            nc.vector.memset(M[:, :, 0], 0.0)

            # phase_end cols 0,1: bit-exact replica of XLA-CPU fp32 path
            nc.vector.tensor_scalar(out=t0, in0=f0t, scalar1=1.0,
                                    scalar2=TWO_PI, op0=OP.max, op1=OP.mult)
            for osc, pecol in ((0, 0), (1, 1)):
                rt = r1 if osc == 0 else r2
                stcol = ps4[:, :, 0] if osc == 0 else ps4[:, :, 1]
                nc.vector.tensor_mul(st_t, t0, rt)
                nc.vector.tensor_scalar(out=st_t, in0=st_t,
                                        scalar1=INV_SR, scalar2=None, op0=OP.mult)
                nc.vector.scalar_tensor_tensor(out=wtmp, in0=st_t,
                                               scalar=64.0, in1=stcol,
                                               op0=OP.mult, op1=OP.add)
                nc.vector.tensor_scalar(out=st_t, in0=wtmp,
                                        scalar1=0.5, scalar2=1.0 / TWO_PI,
                                        op0=OP.add, op1=OP.mult)
                nc.vector.tensor_scalar(out=st_t, in0=st_t,
                                        scalar1=-0.5, scalar2=None, op0=OP.add)
                nc.vector.tensor_copy(ktmp, st_t)
                nc.vector.scalar_tensor_tensor(out=pe4[:, :, pecol], in0=ktmp,
                                               scalar=-TWO_PI, in1=wtmp,
                                               op0=OP.mult, op1=OP.add)
            AUD = big_pool.tile([P, F, S], F16, tag=f"AUD_{c}")
            st.update(INC=INC, FB=FB, D2=D2, V=V, M=M, pe4=pe4, ss=ss, rr=rr,
                      ps4=ps4, AUD=AUD, PARAMS=PARAMS)
            return st

        def iter_ops(st, t):
            V, M, INC, FB, D2, AUD = (st["V"], st["M"], st["INC"], st["FB"],
                                      st["D2"], st["AUD"])
            SCR = scr_pool.tile([P, F, 8], FP, tag="SCR")
            U, UF = SCR[:, :, 0:2], SCR[:, :, 4:6]
            K = SCR[:, :, 2:4].bitcast(I32)
            O1 = SCR[:, :, 6]
            MT = st["PARAMS"][:, :, 4:6]    # [M1, T2] fused output
            FD2 = st["PARAMS"][:, :, 1:3]   # [fb1, d2] fused coeffs
            T2 = st["PARAMS"][:, :, 5]
            nc.vector.tensor_add(U, V, M)
            nc.scalar.activation(K, U, AF.Copy)             # RN int cast on ACT
            nc.vector.scalar_tensor_tensor(out=UF, in0=K, scalar=-1.0,
                                           in1=U, op0=OP.mult, op1=OP.add)
            if t < S:   # osc1 does step t -> o1 scratch (fp32); sin-b first:
                nc.scalar.activation(O1, UF[:, :, 1], AF.Sin, scale=TWO_PI)
            if t >= 1:  # osc2 does step t-1 -> audio col t-1 (fp16)
                nc.scalar.activation(AUD[:, :, t - 1], UF[:, :, 0], AF.Sin,
                                     scale=TWO_PI)
            if t < S:
                # fused [M1, T2] = [fb1, d2] * o1 (broadcast o1 over pair)
                nc.vector.tensor_mul(MT, FD2,
                                     O1.unsqueeze(2).broadcast_to((P, F, 2)))
                if t == 0:
                    nc.vector.tensor_mul(M[:, :, 0], FB[:, :, 0],
                                         st["ps4"][:, :, 3])
                else:
                    nc.vector.tensor_mul(M[:, :, 0], FB[:, :, 0],
                                         AUD[:, :, t - 1])
                nc.vector.tensor_add(M[:, :, 0], M[:, :, 0], T2)
                if t == S - 1:
                    nc.vector.tensor_copy(st["pe4"][:, :, 2], O1)
            if t == S:
                nc.vector.tensor_copy(st["pe4"][:, :, 3], AUD[:, :, S - 1])

        def finish(st, c):
            lo, hi, pe4 = st["lo"], st["hi"], st["pe4"]
            AUD, ss, rr = st["AUD"], st["ss"], st["rr"]
            nc.scalar.dma_start(
                out=pe_out[lo:hi, :].rearrange("(p f) c -> p f c", p=P), in_=pe4[:]
            )
            for b in range(NBLK):
                SQb = big_pool.tile([P, F, SBK], FP, tag="SQb")
                asl = AUD[:, :, b * SBK : (b + 1) * SBK]
                nc.gpsimd.tensor_tensor(SQb[:], asl, asl, OP.mult)
                ssb = scr_pool.tile([P, F], FP, tag="ssb")
                nc.vector.reduce_sum(ssb[:], SQb[:], axis=mybir.AxisListType.X)
                if b == 0:
                    nc.vector.tensor_copy(ss, ssb[:])
                else:
                    nc.vector.tensor_add(ss, ss, ssb[:])
            # rr = 1/sqrt(ss/64 + 1e-5)
            nc.vector.tensor_scalar(out=ss, in0=ss, scalar1=1.0 / S,
                                    scalar2=1e-5, op0=OP.mult, op1=OP.add)
            nc.scalar.activation(ss, ss, AF.Sqrt)
            nc.vector.reciprocal(rr, ss)
            nc.gpsimd.tensor_tensor(
                AUD[:], AUD[:], rr.unsqueeze(2).broadcast_to((P, F, S)), OP.mult
            )
            # cast fp16 -> fp32 on the way out (SWDGE cast DMA)
            nc.gpsimd.dma_start(
                out=audio_out[lo:hi, :].rearrange("(p f) s -> p f s", p=P),
                in_=AUD[:],
            )

        sts = [setup(c) for c in range(NCHUNK)]
        for t in range(S + 1):
            # fused V += INC for both chunks (off the serial chain)
            nc.vector.tensor_add(VINC[:, :, 4:8], VINC[:, :, 4:8],
                                 VINC[:, :, 0:4])
            for st in sts:
                iter_ops(st, t)
        for c, st in enumerate(sts):
            finish(st, c)

    split_multi_waits(nc)
    return nc


_NC = None


def kernel(fm_params: np.ndarray, f0_hz: np.ndarray, phase_state: np.ndarray):
    global _NC
    if _NC is None:
        _NC = _build()
    fm = np.ascontiguousarray(fm_params.reshape(B, 6), dtype=np.float32)
    f0 = np.ascontiguousarray(f0_hz.reshape(B, 1), dtype=np.float32)
    ps = np.ascontiguousarray(phase_state.reshape(B, 4), dtype=np.float32)
    in_maps = [
        {
            "fm": fm[i * VC : (i + 1) * VC],
            "f0": f0[i * VC : (i + 1) * VC],
            "ps": ps[i * VC : (i + 1) * VC],
        }
        for i in range(NCORES)
    ]
    res = None
    for attempt in range(3):
        try:
            res = run_bass_kernel_spmd(_NC, in_maps, list(range(NCORES))).results
            break
        except Exception:
            # transient NRT_EXEC_UNIT_UNRECOVERABLE wedges happen; retry
            if attempt == 2:
                raise
            import time as _time
            _time.sleep(5)
    audio = np.concatenate([r["audio"] for r in res], axis=0)
    pe = np.concatenate([r["pe"] for r in res], axis=0)
    return audio, pe


# revision 22
# speedup vs baseline: 1.0431x; 1.0431x over previous
"""FM synth (2-op feedback FM) Trainium2 kernel.

Contract: kernel(**inputs) takes FULL unsharded inputs
  fm_params [524288, 1, 6] f32, f0_hz [524288, 1] f32, phase_state [524288, 4] f32
returns (audio [524288, 64] f32, phase_end [524288, 4] f32), matching reference.

Data-parallel over 8 NeuronCores (batch sharding). Per core: 65536 voices.

Math (per voice, 64 steps, all phases tracked in TURNS = radians/2pi):
  inc1 = f0c*r1/16000, inc2 = f0c*r2/16000   (turns/step)
  v1_t = start1' + (t+1)*inc1 ; v2 lags one step behind (skewed fusion)
  osc1 step t:   u1 = v1_t + m1;  o1 = sin(2*pi*wrap(u1));  m1' = fb1'*o1
  osc2 step t-1: u2 = v2_t + m2;  o2 = sin(2*pi*wrap(u2));
                 m2' = fb2'*o2 + d2'*o1_t
  wrap(u) = u - round_to_nearest_int(u)  (int32 cast is RN on DVE)
  audio[v, t-1] = o2 ; then RMS-normalize over the 64 samples.

Layout: interleaved oscillator pairs [128 partitions, F voices, 2] with
c=0 -> osc2, c=1 -> osc1, so one [128, 2F] instruction covers both oscs and
the ACT sin writes osc2 into its audio column and osc1 into a scratch
column of the same tile via one affine access pattern
(audio tile is [128, F, 65]: columns 0..63 audio, column 64 = o1 scratch).
"""
import sys, math
from contextlib import ExitStack

import numpy as np

sys.path.insert(0, "/opt/trn_rl_repo")

import concourse.bass as bass
import concourse.tile as tile
from concourse import mybir
from concourse.bass_utils import run_bass_kernel_spmd

FP = mybir.dt.float32
I32 = mybir.dt.int32
AF = mybir.ActivationFunctionType
OP = mybir.AluOpType

B = 524288
NCORES = 8
VC = B // NCORES          # 65536 voices per core
S = 64                    # samples per block
P = 128                   # partitions
F = 256                   # voices per partition per chunk
CHUNK = P * F             # 32768 voices per chunk
NCHUNK = VC // CHUNK      # 2 chunks per core
TWO_PI = 2.0 * math.pi
INV_SR = 1.0 / 16000.0


def split_multi_waits(nc, max_waits: int = 1):
    """This walrus build rejects >1 sync-wait on most instruction forms.
    Split extras into standalone InstEventSemaphore waits inserted before."""
    k = 0
    for f in nc.m.functions:
        for blk in f.blocks:
            out = []
            for inst in blk.instructions:
                si = inst.sync_info
                if si is not None and len(si.on_wait) > max_waits:
                    waits = list(si.on_wait)
                    for w in waits[:-max_waits]:
                        out.append(
                            mybir.InstEventSemaphore(
                                name=f"splitw-{k}",
                                engine=inst.engine,
                                ins=[],
                                outs=[],
                                sync_info=mybir.SyncInfo(on_wait=[w], on_update=[]),
                            )
                        )
                        k += 1
                    inst.sync_info = mybir.SyncInfo(
                        on_wait=waits[-max_waits:], on_update=list(si.on_update)
                    )
                out.append(inst)
            blk.instructions = out
    return k


F16 = mybir.dt.float16
SBK = 8                    # sample-block size for RMS squaring
NBLK = S // SBK


def _build():
    nc = bass.Bass()
    fm_in = nc.declare_dram_parameter("fm", [VC, 6], FP, isOutput=False)
    f0_in = nc.declare_dram_parameter("f0", [VC, 1], FP, isOutput=False)
    ps_in = nc.declare_dram_parameter("ps", [VC, 4], FP, isOutput=False)
    audio_out = nc.declare_dram_parameter("audio", [VC, S], FP, isOutput=True)
    pe_out = nc.declare_dram_parameter("pe", [VC, 4], FP, isOutput=True)

    with ExitStack() as ctx:
        tc = ctx.enter_context(tile.TileContext(nc))
        io_pool = ctx.enter_context(tc.tile_pool(name="io", bufs=2))
        par_pool = ctx.enter_context(tc.tile_pool(name="par", bufs=1))
        one_pool = ctx.enter_context(tc.tile_pool(name="one", bufs=1))
        big_pool = ctx.enter_context(tc.tile_pool(name="big", bufs=1))
        scr_pool = ctx.enter_context(tc.tile_pool(name="scr", bufs=3))

        VINC = par_pool.tile([P, F, 8], FP, tag="VINC")

        def setup(c):
            lo = c * CHUNK
            hi = lo + CHUNK
            st = {"lo": lo, "hi": hi, "c": c}
            # VINC shared across chunks so the per-step V += INC fuses into
            # one wide op; FB/M per chunk (pairs: 0=osc2, 1=osc1)
            INC = VINC[:, :, 2 * c : 2 * c + 2]
            V = VINC[:, :, 4 + 2 * c : 6 + 2 * c]
            # cols: fb2 0 | fb1 1 | d2 2 | M2 3 | M1 4 | T2 5  (so that
            # [fb1,d2] and [M2,M1] and [M1,T2] are contiguous pair-slices)
            PARAMS = par_pool.tile([P, F, 8], FP, tag=f"PARAMS_{c}")
            FB = PARAMS[:, :, 0:2]          # [fb2, fb1]
            M = PARAMS[:, :, 3:5]           # [M2, M1] (0=osc2, 1=osc1)
            PARC = par_pool.tile([P, F, 4], FP, tag=f"PARC_{c}")
            ss, rr = PARC[:, :, 1], PARC[:, :, 2]
            D2 = PARAMS[:, :, 2]
            ps4 = par_pool.tile([P, F, 4], FP, tag=f"ps4_{c}")
            pe4 = par_pool.tile([P, F, 4], FP, tag=f"pe4_{c}")
            ONE = one_pool.tile([P, F, 12], FP, tag="ONE")
            f0t, f0s, r1, r2 = ONE[:, :, 0], ONE[:, :, 1], ONE[:, :, 2], ONE[:, :, 3]
            s1t, s2t, t0 = ONE[:, :, 4], ONE[:, :, 5], ONE[:, :, 6]
            wtmp, st_t = ONE[:, :, 7], ONE[:, :, 8]
            ktmp = ONE[:, :, 9:10].bitcast(I32)[:, :, 0]
            fm6 = io_pool.tile([P, F, 6], FP, tag="fm6")
            f0ld = io_pool.tile([P, F], FP, tag="f0ld")

            nc.gpsimd.dma_start(
                out=fm6[:], in_=fm_in[lo:hi, :].rearrange("(p f) c -> p f c", p=P)
            )
            nc.gpsimd.dma_start(
                out=ps4[:], in_=ps_in[lo:hi, :].rearrange("(p f) c -> p f c", p=P)
            )
            nc.gpsimd.dma_start(
                out=f0ld[:], in_=f0_in[lo:hi, 0].rearrange("(p f) -> p f", p=P)
            )
            nc.vector.tensor_copy(f0t, f0ld[:])

            nc.vector.tensor_scalar(out=f0s, in0=f0t, scalar1=1.0,
                                    scalar2=INV_SR, op0=OP.max, op1=OP.mult)
            nc.vector.tensor_scalar(out=r1, in0=fm6[:, :, 1], scalar1=15.75,
                                    scalar2=0.25, op0=OP.mult, op1=OP.add)
            nc.vector.tensor_scalar(out=r2, in0=fm6[:, :, 4], scalar1=15.75,
                                    scalar2=0.25, op0=OP.mult, op1=OP.add)

            nc.vector.tensor_mul(INC[:, :, 1], f0s, r1)
            nc.vector.tensor_mul(INC[:, :, 0], f0s, r2)
            nc.vector.tensor_scalar(out=FB[:, :, 1], in0=fm6[:, :, 2],
                                    scalar1=0.95 / TWO_PI, scalar2=None, op0=OP.mult)
            nc.vector.tensor_scalar(out=FB[:, :, 0], in0=fm6[:, :, 5],
                                    scalar1=0.95 / TWO_PI, scalar2=None, op0=OP.mult)
            nc.vector.tensor_scalar(out=D2, in0=fm6[:, :, 3],
                                    scalar1=10.0 / TWO_PI, scalar2=None, op0=OP.mult)

            # V init; loop does V += INC first, so pre-subtract
            nc.vector.tensor_scalar(out=s1t, in0=ps4[:, :, 0],
                                    scalar1=1.0 / TWO_PI, scalar2=None, op0=OP.mult)
            nc.vector.scalar_tensor_tensor(out=V[:, :, 1], in0=INC[:, :, 1],
                                           scalar=-1.0, in1=s1t,
                                           op0=OP.mult, op1=OP.add)
            nc.vector.tensor_scalar(out=s2t, in0=ps4[:, :, 1],
                                    scalar1=1.0 / TWO_PI, scalar2=None, op0=OP.mult)
            nc.vector.scalar_tensor_tensor(out=V[:, :, 0], in0=INC[:, :, 0],
                                           scalar=-2.0, in1=s2t,
                                           op0=OP.mult, op1=OP.add)

            nc.vector.tensor_mul(M[:, :, 1], FB[:, :, 1], ps4[:, :, 2])
            nc.vector.memset(M[:, :, 0], 0.0)

            # phase_end cols 0,1: bit-exact replica of XLA-CPU fp32 path
            nc.vector.tensor_scalar(out=t0, in0=f0t, scalar1=1.0,
                                    scalar2=TWO_PI, op0=OP.max, op1=OP.mult)
            for osc, pecol in ((0, 0), (1, 1)):
                rt = r1 if osc == 0 else r2
                stcol = ps4[:, :, 0] if osc == 0 else ps4[:, :, 1]
                nc.vector.tensor_mul(st_t, t0, rt)
                nc.vector.tensor_scalar(out=st_t, in0=st_t,
                                        scalar1=INV_SR, scalar2=None, op0=OP.mult)
                nc.vector.scalar_tensor_tensor(out=wtmp, in0=st_t,
                                               scalar=64.0, in1=stcol,
                                               op0=OP.mult, op1=OP.add)
                nc.vector.tensor_scalar(out=st_t, in0=wtmp,
                                        scalar1=0.5, scalar2=1.0 / TWO_PI,
                                        op0=OP.add, op1=OP.mult)
                nc.vector.tensor_scalar(out=st_t, in0=st_t,
                                        scalar1=-0.5, scalar2=None, op0=OP.add)
                nc.vector.tensor_copy(ktmp, st_t)
                nc.vector.scalar_tensor_tensor(out=pe4[:, :, pecol], in0=ktmp,
                                               scalar=-TWO_PI, in1=wtmp,
                                               op0=OP.mult, op1=OP.add)
            AUD = big_pool.tile([P, F, S], F16, tag=f"AUD_{c}")
            st.update(INC=INC, FB=FB, D2=D2, V=V, M=M, pe4=pe4, ss=ss, rr=rr,
                      ps4=ps4, AUD=AUD, PARAMS=PARAMS)
            return st

        def stage_u(st, t):
            SCR = scr_pool.tile([P, F, 8], FP, tag="SCR")
            st["SCR"] = SCR
            nc.vector.tensor_add(SCR[:, :, 0:2], st["V"], st["M"])

        def stage_cast(st, t):
            SCR = st["SCR"]
            nc.scalar.activation(SCR[:, :, 2:4].bitcast(I32), SCR[:, :, 0:2],
                                 AF.Copy)                   # RN int cast on ACT

        def stage_uf(st, t):
            SCR = st["SCR"]
            nc.vector.scalar_tensor_tensor(
                out=SCR[:, :, 4:6], in0=SCR[:, :, 2:4].bitcast(I32),
                scalar=-1.0, in1=SCR[:, :, 0:2], op0=OP.mult, op1=OP.add)

        def stage_sin(st, t):
            SCR, AUD = st["SCR"], st["AUD"]
            UF = SCR[:, :, 4:6]
            if t < S:   # osc1 step t -> o1 scratch (fp32); chain-first
                nc.scalar.activation(SCR[:, :, 6], UF[:, :, 1], AF.Sin,
                                     scale=TWO_PI)
            if t >= 1:  # osc2 step t-1 -> audio col t-1 (fp16)
                nc.scalar.activation(AUD[:, :, t - 1], UF[:, :, 0], AF.Sin,
                                     scale=TWO_PI)

        def stage_m(st, t):
            SCR, AUD, M, FB = st["SCR"], st["AUD"], st["M"], st["FB"]
            O1 = SCR[:, :, 6]
            MT = st["PARAMS"][:, :, 4:6]    # [M1, T2] fused output
            FD2 = st["PARAMS"][:, :, 1:3]   # [fb1, d2] fused coeffs
            T2 = st["PARAMS"][:, :, 5]
            if t < S:
                nc.vector.tensor_mul(MT, FD2,
                                     O1.unsqueeze(2).broadcast_to((P, F, 2)))
                if t == 0:
                    nc.vector.tensor_mul(M[:, :, 0], FB[:, :, 0],
                                         st["ps4"][:, :, 3])
                else:
                    nc.vector.tensor_mul(M[:, :, 0], FB[:, :, 0],
                                         AUD[:, :, t - 1])
                nc.vector.tensor_add(M[:, :, 0], M[:, :, 0], T2)
                if t == S - 1:
                    nc.vector.tensor_copy(st["pe4"][:, :, 2], O1)
            if t == S:
                nc.vector.tensor_copy(st["pe4"][:, :, 3], AUD[:, :, S - 1])

        def finish(st, c):
            lo, hi, pe4 = st["lo"], st["hi"], st["pe4"]
            AUD, ss, rr = st["AUD"], st["ss"], st["rr"]
            nc.scalar.dma_start(
                out=pe_out[lo:hi, :].rearrange("(p f) c -> p f c", p=P), in_=pe4[:]
            )
            for b in range(NBLK):
                SQb = scr_pool.tile([P, F, SBK], FP, tag="SQb")
                asl = AUD[:, :, b * SBK : (b + 1) * SBK]
                nc.gpsimd.tensor_tensor(SQb[:], asl, asl, OP.mult)
                ssb = scr_pool.tile([P, F], FP, tag="ssb")
                nc.vector.reduce_sum(ssb[:], SQb[:], axis=mybir.AxisListType.X)
                if b == 0:
                    nc.vector.tensor_copy(ss, ssb[:])
                else:
                    nc.vector.tensor_add(ss, ss, ssb[:])
            # rr = 1/sqrt(ss/64 + 1e-5)
            nc.vector.tensor_scalar(out=ss, in0=ss, scalar1=1.0 / S,
                                    scalar2=1e-5, op0=OP.mult, op1=OP.add)
            nc.scalar.activation(ss, ss, AF.Sqrt)
            nc.vector.reciprocal(rr, ss)
            nc.gpsimd.tensor_tensor(
                AUD[:], AUD[:], rr.unsqueeze(2).broadcast_to((P, F, S)), OP.mult
            )
            # cast fp16 -> fp32 on the way out (SWDGE cast DMA)
            nc.gpsimd.dma_start(
                out=audio_out[lo:hi, :].rearrange("(p f) s -> p f s", p=P),
                in_=AUD[:],
            )

        sts = [setup(c) for c in range(NCHUNK)]
        for t in range(S + 1):
            # fused V += INC for both chunks (off the serial chain)
            nc.vector.tensor_add(VINC[:, :, 4:8], VINC[:, :, 4:8],
                                 VINC[:, :, 0:4])
            # stage-interleaved emission: ping-pong priority across streams
            for stage in (stage_u, stage_cast, stage_uf, stage_sin, stage_m):
                for st in sts:
                    stage(st, t)
        for c, st in enumerate(sts):
            finish(st, c)

    split_multi_waits(nc)
    return nc


_NC = None


def kernel(fm_params: np.ndarray, f0_hz: np.ndarray, phase_state: np.ndarray):
    global _NC
    if _NC is None:
        _NC = _build()
    fm = np.ascontiguousarray(fm_params.reshape(B, 6), dtype=np.float32)
    f0 = np.ascontiguousarray(f0_hz.reshape(B, 1), dtype=np.float32)
    ps = np.ascontiguousarray(phase_state.reshape(B, 4), dtype=np.float32)
    in_maps = [
        {
            "fm": fm[i * VC : (i + 1) * VC],
            "f0": f0[i * VC : (i + 1) * VC],
            "ps": ps[i * VC : (i + 1) * VC],
        }
        for i in range(NCORES)
    ]
    res = None
    for attempt in range(3):
        try:
            res = run_bass_kernel_spmd(_NC, in_maps, list(range(NCORES))).results
            break
        except Exception:
            # transient NRT_EXEC_UNIT_UNRECOVERABLE wedges happen; retry
            if attempt == 2:
                raise
            import time as _time
            _time.sleep(5)
    audio = np.concatenate([r["audio"] for r in res], axis=0)
    pe = np.concatenate([r["pe"] for r in res], axis=0)
    return audio, pe
